# revision 11
# baseline (speedup 1.0000x reference)
"""Trainium2 Bass kernel for a serialized-attention transformer block.

Strategy (8 NeuronCores, data-parallel over serialized patches):
  host: fold LoRA + LN affine into weights, gather rows into serialized
        order, transpose to feature-major [C, rows], shard rows 8 ways.
  device (per core, rows R=8192, all activations feature-major):
        LN1 -> qkv -> per-patch attention (128-row patches) -> proj
        -> residual -> LN2 -> mlp (gelu-tanh) -> residual.
  host: transpose back and scatter rows to original order.

LN statistics are computed with ones-matmuls on the tensor engine
(partition-axis reductions); per-row scalars are broadcast across
partitions with K=1 matmuls.  Attention uses transposed scores
(lhsT=k, rhs=q) so softmax denominators come from a ones-matmul and no
PE transposes are needed anywhere.
"""

import os
import sys

import numpy as np

if "/opt/trn_rl_repo" not in sys.path:
    sys.path.insert(0, "/opt/trn_rl_repo")

N, C, H, K, R = 65536, 512, 8, 128, 16
D = C // H
HID = 4 * C
LORA_SCALE = 32.0 / 16.0
SCALE = D**-0.5
NCORES = 8
RPC = N // NCORES          # rows per core
RT = 512                   # rows per tile (4 patches)
NRT = RPC // RT
PPT = RT // K              # patches per row-tile
CCH = C // 128             # feature chunks of x (4)
QKCH = 8                   # q+k feature chunks
HCH = HID // 128           # hidden chunks (16)
F32 = None                 # set at build time (mybir.dt.float32)

_STATE = {}


def _build():
    import concourse.tile as tile
    from concourse import bacc, mybir

    nrt = int(os.environ.get("KERNEL_NRT", NRT))
    passes = os.environ.get("KERNEL_PASSES", "AB")
    f32 = mybir.dt.float32
    nc = bacc.Bacc(None, target_bir_lowering=False, debug=False)

    xin = nc.dram_tensor("xin", [C, RPC], f32, kind="ExternalInput")
    yout = nc.dram_tensor("yout", [C, RPC], f32, kind="ExternalOutput")
    f2d = nc.dram_tensor("feat2", [C, RPC], f32, kind="Internal")

    wqkv = nc.dram_tensor("wqkv", [128, CCH, 3 * C], f32, kind="ExternalInput")
    bqkv = nc.dram_tensor("bqkv", [128, 12], f32, kind="ExternalInput")
    bvbc = nc.dram_tensor("bvbc", [C], f32, kind="ExternalInput")
    wproj = nc.dram_tensor("wproj", [128, CCH, C], f32, kind="ExternalInput")
    bproj = nc.dram_tensor("bproj", [128, CCH], f32, kind="ExternalInput")
    w1 = nc.dram_tensor("w1", [128, CCH, HID], f32, kind="ExternalInput")
    b1h = nc.dram_tensor("b1h", [128, HCH], f32, kind="ExternalInput")
    w2 = nc.dram_tensor("w2", [128, HCH, C], f32, kind="ExternalInput")
    b2o = nc.dram_tensor("b2o", [128, CCH], f32, kind="ExternalInput")

    import concourse.bass as bass

    xin_r = xin[:].rearrange("(c p) r -> p c r", p=128)
    yout_r = yout[:].rearrange("(c p) r -> p c r", p=128)
    f2d_r = f2d[:].rearrange("(c p) r -> p c r", p=128)

    with tile.TileContext(nc) as tc:
        with (
            tc.tile_pool(name="const", bufs=1) as constp,
            tc.tile_pool(name="psum", bufs=1, space="PSUM") as psp,
        ):
            ones128 = constp.tile([128, 1], f32)
            nc.vector.memset(ones128, 1.0)
            invC = constp.tile([128, 1], f32)
            nc.vector.memset(invC, 1.0 / C)
            ones_row = constp.tile([1, 128], f32)
            nc.vector.memset(ones_row, 1.0)
            epsb = constp.tile([128, 1], f32)
            nc.vector.memset(epsb, 1e-5)

            def layernorm(sb, x, xh):
                """x: [128, CCH, RT] sbuf -> xh normalized (no affine)."""
                x2 = sb.tile([128, CCH, RT], f32, tag="x2", bufs=1)
                nc.vector.tensor_mul(x2[:], x[:], x[:])
                s1 = psp.tile([1, RT], f32, tag="pss", bufs=2)
                s2 = psp.tile([1, RT], f32, tag="pss", bufs=2)
                for c in range(CCH):
                    nc.tensor.matmul(
                        s1[:], invC[:], x[:, c, :], start=(c == 0), stop=(c == CCH - 1)
                    )
                for c in range(CCH):
                    nc.tensor.matmul(
                        s2[:], invC[:], x2[:, c, :], start=(c == 0), stop=(c == CCH - 1)
                    )
                s1b = sb.tile([1, RT], f32, tag="s1b", bufs=2)
                s2b = sb.tile([1, RT], f32, tag="s2b", bufs=2)
                nc.scalar.copy(s1b[:], s1[:])
                nc.scalar.copy(s2b[:], s2[:])
                mb = psp.tile([128, RT], f32, tag="psb", bufs=6)
                m2b = psp.tile([128, RT], f32, tag="psb", bufs=6)
                nc.tensor.matmul(mb[:], ones_row[:], s1b[:], start=True, stop=True)
                nc.tensor.matmul(m2b[:], ones_row[:], s2b[:], start=True, stop=True)
                var = sb.tile([128, RT], f32, tag="var", bufs=2)
                nc.scalar.square(var[:], mb[:])
                nc.vector.tensor_sub(var[:], m2b[:], var[:])
                sd = sb.tile([128, RT], f32, tag="sd", bufs=2)
                nc.scalar.activation(
                    sd[:], var[:], mybir.ActivationFunctionType.Sqrt, bias=epsb[:]
                )
                ab = sb.tile([128, RT], f32, tag="ab", bufs=2)
                nc.vector.reciprocal(ab[:], sd[:])
                mbb = mb[:, None, :].to_broadcast([128, CCH, RT])
                abb = ab[:, None, :].to_broadcast([128, CCH, RT])
                nc.vector.tensor_sub(xh[:], x[:], mbb)
                nc.vector.tensor_mul(xh[:], xh[:], abb)

            # ---------------- pass A: attention block ----------------
            if "A" not in passes:
                nrt_a = 0
            else:
                nrt_a = nrt
            with (
                tc.tile_pool(name="wA", bufs=1) as wp,
                tc.tile_pool(name="sbA", bufs=1) as sb,
            ):
                wqkv_sb = wp.tile([128, CCH, 3 * C], f32)
                nc.sync.dma_start(wqkv_sb[:], wqkv[:])
                bqkv_sb = wp.tile([128, 12], f32)
                nc.sync.dma_start(bqkv_sb[:], bqkv[:])
                wproj_sb = wp.tile([128, CCH, C], f32)
                nc.sync.dma_start(wproj_sb[:], wproj[:])
                bproj_sb = wp.tile([128, CCH], f32)
                nc.sync.dma_start(bproj_sb[:], bproj[:])
                bv_sb = wp.tile([128, C], f32)
                nc.sync.dma_start(
                    bv_sb[:],
                    bass.AP(tensor=bvbc, offset=0, ap=[[0, 128], [1, C]]),
                )

                for rt in range(nrt_a):
                    rsl = slice(rt * RT, (rt + 1) * RT)
                    x = sb.tile([128, CCH, RT], f32, tag="x", bufs=2)
                    nc.sync.dma_start(x[:], xin_r[:, :, rsl])
                    xh = sb.tile([128, CCH, RT], f32, tag="xh", bufs=2)
                    layernorm(sb, x, xh)

                    # q, k (feature-major) with bias
                    q = sb.tile([128, CCH, RT], f32, tag="q", bufs=2)
                    k = sb.tile([128, CCH, RT], f32, tag="k", bufs=2)
                    for fc in range(QKCH):
                        ps = psp.tile([128, RT], f32, tag="psb", bufs=6)
                        for c in range(CCH):
                            nc.tensor.matmul(
                                ps[:],
                                wqkv_sb[:, c, fc * 128 : (fc + 1) * 128],
                                xh[:, c, :],
                                start=(c == 0),
                                stop=(c == CCH - 1),
                            )
                        dst = q if fc < CCH else k
                        nc.vector.tensor_scalar(
                            dst[:, fc % CCH, :],
                            ps[:],
                            bqkv_sb[:, fc : fc + 1],
                            None,
                            mybir.AluOpType.add,
                        )
                    # v (row-major per patch) with bias
                    v = sb.tile([128, PPT, H, D], f32, tag="v", bufs=2)
                    for pi in range(PPT):
                        psl = slice(pi * K, (pi + 1) * K)
                        psv = psp.tile([128, C], f32, tag="psb", bufs=6)
                        for c in range(CCH):
                            nc.tensor.matmul(
                                psv[:],
                                xh[:, c, psl],
                                wqkv_sb[:, c, 2 * C : 3 * C],
                                start=(c == 0),
                                stop=(c == CCH - 1),
                            )
                        nc.vector.tensor_add(
                            v[:, pi, :, :].rearrange("p h d -> p (h d)"),
                            psv[:],
                            bv_sb[:],
                        )

                    # attention per patch
                    o = sb.tile([128, CCH, PPT, K], f32, tag="o", bufs=2)
                    for pi in range(PPT):
                        psl = slice(pi * K, (pi + 1) * K)
                        sa = psp.tile([128, CCH, K], f32, tag="psb", bufs=6)
                        sbp = psp.tile([128, CCH, K], f32, tag="psb", bufs=6)
                        for j in range(CCH):
                            nc.tensor.matmul(
                                sa[:, j, :],
                                k[0:64, j, psl],
                                q[0:64, j, psl],
                                start=True,
                                stop=True,
                            )
                            nc.tensor.matmul(
                                sbp[:, j, :],
                                k[64:128, j, psl],
                                q[64:128, j, psl],
                                start=True,
                                stop=True,
                            )
                        ea = sb.tile([128, CCH, K], f32, tag="ea", bufs=2)
                        eb = sb.tile([128, CCH, K], f32, tag="eb", bufs=2)
                        nc.scalar.activation(
                            ea[:], sa[:], mybir.ActivationFunctionType.Exp
                        )
                        nc.scalar.activation(
                            eb[:], sbp[:], mybir.ActivationFunctionType.Exp
                        )
                        sua = psp.tile([1, RT], f32, tag="pss", bufs=2)
                        sub = psp.tile([1, RT], f32, tag="pss", bufs=2)
                        nc.tensor.matmul(
                            sua[:], ones128[:], ea[:].rearrange("p c r -> p (c r)"),
                            start=True, stop=True,
                        )
                        nc.tensor.matmul(
                            sub[:], ones128[:], eb[:].rearrange("p c r -> p (c r)"),
                            start=True, stop=True,
                        )
                        ra = sb.tile([1, RT], f32, tag="ra", bufs=2)
                        rb = sb.tile([1, RT], f32, tag="rb", bufs=2)
                        nc.vector.reciprocal(ra[:], sua[:])
                        nc.vector.reciprocal(rb[:], sub[:])
                        rba = psp.tile([128, CCH, K], f32, tag="psb", bufs=6)
                        rbb = psp.tile([128, CCH, K], f32, tag="psb", bufs=6)
                        nc.tensor.matmul(
                            rba[:].rearrange("p c r -> p (c r)"), ones_row[:], ra[:],
                            start=True, stop=True,
                        )
                        nc.tensor.matmul(
                            rbb[:].rearrange("p c r -> p (c r)"), ones_row[:], rb[:],
                            start=True, stop=True,
                        )
                        ops = psp.tile([128, CCH, K], f32, tag="psb", bufs=6)
                        for j in range(CCH):
                            nc.tensor.matmul(
                                ops[0:64, j, :],
                                v[:, pi, 2 * j, :],
                                ea[:, j, :],
                                start=True,
                                stop=True,
                            )
                            nc.tensor.matmul(
                                ops[64:128, j, :],
                                v[:, pi, 2 * j + 1, :],
                                eb[:, j, :],
                                start=True,
                                stop=True,
                            )
                        nc.scalar.copy(o[:, :, pi, :], ops[:, :, :])
                        nc.vector.tensor_mul(
                            o[0:64, :, pi, :], o[0:64, :, pi, :], rba[0:64, :, :]
                        )
                        nc.vector.tensor_mul(
                            o[64:128, :, pi, :], o[64:128, :, pi, :], rbb[64:128, :, :]
                        )

                    # proj + residual -> feat2
                    f2 = sb.tile([128, CCH, RT], f32, tag="f2", bufs=2)
                    for c in range(CCH):
                        ps = psp.tile([128, RT], f32, tag="psb", bufs=6)
                        for cc in range(CCH):
                            nc.tensor.matmul(
                                ps[:],
                                wproj_sb[:, cc, c * 128 : (c + 1) * 128],
                                o[:, cc, :, :].rearrange("p t r -> p (t r)"),
                                start=(cc == 0),
                                stop=(cc == CCH - 1),
                            )
                        nc.vector.tensor_scalar(
                            f2[:, c, :],
                            ps[:],
                            bproj_sb[:, c : c + 1],
                            None,
                            mybir.AluOpType.add,
                        )
                        nc.vector.tensor_add(f2[:, c, :], f2[:, c, :], x[:, c, :])
                    nc.sync.dma_start(f2d_r[:, :, rsl], f2[:])

            # ---------------- pass B: MLP block ----------------
            if "B" not in passes:
                nrt_b = 0
            else:
                nrt_b = nrt
            with (
                tc.tile_pool(name="wB", bufs=1) as wp,
                tc.tile_pool(name="sbB", bufs=1) as sb,
            ):
                w1_sb = wp.tile([128, CCH, HID], f32)
                nc.sync.dma_start(w1_sb[:], w1[:])
                b1h_sb = wp.tile([128, HCH], f32)
                nc.sync.dma_start(b1h_sb[:], b1h[:])
                w2_sb = wp.tile([128, HCH, C], f32)
                nc.sync.dma_start(w2_sb[:], w2[:])
                b2o_sb = wp.tile([128, CCH], f32)
                nc.sync.dma_start(b2o_sb[:], b2o[:])

                for rt in range(nrt_b):
                    rsl = slice(rt * RT, (rt + 1) * RT)
                    x = sb.tile([128, CCH, RT], f32, tag="x", bufs=2)
                    nc.sync.dma_start(x[:], f2d_r[:, :, rsl])
                    xh = sb.tile([128, CCH, RT], f32, tag="xh", bufs=2)
                    layernorm(sb, x, xh)

                    h = sb.tile([128, HCH, RT], f32, tag="h", bufs=1)
                    for fc in range(HCH):
                        ps = psp.tile([128, RT], f32, tag="psb", bufs=6)
                        for c in range(CCH):
                            nc.tensor.matmul(
                                ps[:],
                                w1_sb[:, c, fc * 128 : (fc + 1) * 128],
                                xh[:, c, :],
                                start=(c == 0),
                                stop=(c == CCH - 1),
                            )
                        nc.scalar.activation(
                            h[:, fc, :],
                            ps[:],
                            mybir.ActivationFunctionType.Gelu_apprx_tanh,
                            bias=b1h_sb[:, fc : fc + 1],
                        )
                    yo = sb.tile([128, CCH, RT], f32, tag="yo", bufs=2)
                    for c in range(CCH):
                        ps = psp.tile([128, RT], f32, tag="psb", bufs=6)
                        for cc in range(HCH):
                            nc.tensor.matmul(
                                ps[:],
                                w2_sb[:, cc, c * 128 : (c + 1) * 128],
                                h[:, cc, :],
                                start=(cc == 0),
                                stop=(cc == HCH - 1),
                            )
                        nc.vector.tensor_scalar(
                            yo[:, c, :],
                            ps[:],
                            b2o_sb[:, c : c + 1],
                            None,
                            mybir.AluOpType.add,
                        )
                        nc.vector.tensor_add(yo[:, c, :], yo[:, c, :], x[:, c, :])
                    nc.sync.dma_start(yout_r[:, :, rsl], yo[:])

    nc.compile()
    return nc


def _fold_weights(ins):
    """Host-side constant folding: LoRA into base weights, LN affine into
    the following linear layer, attention scale into q columns."""
    g = lambda n: np.asarray(ins[n], np.float32)
    out = {}

    weff = g("Wqkv") + LORA_SCALE * (g("Aqkv") @ g("Bqkv"))
    wq = g("g1")[:, None] * weff
    bq = g("bqkv") + g("b1") @ weff
    wq[:, :C] *= SCALE
    bq = bq.copy()
    bq[:C] *= SCALE
    out["wqkv"] = np.ascontiguousarray(
        wq.reshape(CCH, 128, 3 * C).transpose(1, 0, 2)
    )
    out["bqkv"] = np.ascontiguousarray(bq.reshape(12, 128).T)
    out["bvbc"] = np.ascontiguousarray(bq[2 * C : 3 * C])

    wp = g("Wproj") + LORA_SCALE * (g("Aproj") @ g("Bproj"))
    out["wproj"] = np.ascontiguousarray(wp.reshape(CCH, 128, C).transpose(1, 0, 2))
    out["bproj"] = np.ascontiguousarray(g("bproj").reshape(CCH, 128).T)

    w1eff = g("W1") + LORA_SCALE * (g("A1") @ g("B1"))
    w1f = g("g2")[:, None] * w1eff
    b1f = g("bfc1") + g("b2") @ w1eff
    out["w1"] = np.ascontiguousarray(w1f.reshape(CCH, 128, HID).transpose(1, 0, 2))
    out["b1h"] = np.ascontiguousarray(b1f.reshape(HCH, 128).T)

    w2eff = g("W2") + LORA_SCALE * (g("A2") @ g("B2"))
    out["w2"] = np.ascontiguousarray(w2eff.reshape(HCH, 128, C).transpose(1, 0, 2))
    out["b2o"] = np.ascontiguousarray(g("bfc2").reshape(CCH, 128).T)
    return out


def kernel(**inputs):
    from concourse.bass_utils import run_bass_kernel_spmd

    if "nc" not in _STATE:
        _STATE["nc"] = _build()
    nc = _STATE["nc"]

    feat = np.asarray(inputs["feat"], np.float32)
    order = np.asarray(inputs["order"], np.int64)
    w = _fold_weights(inputs)

    feat_ser = feat[order]  # serialized order
    in_maps = []
    for cid in range(NCORES):
        m = dict(w)
        m["xin"] = np.ascontiguousarray(
            feat_ser[cid * RPC : (cid + 1) * RPC].T
        )
        in_maps.append(m)

    res = run_bass_kernel_spmd(nc, in_maps, core_ids=list(range(NCORES)))
    _STATE["last_result"] = res

    y_ser = np.empty((N, C), np.float32)
    for cid in range(NCORES):
        y_ser[cid * RPC : (cid + 1) * RPC] = res.results[cid]["yout"].T
    out = np.empty((N, C), np.float32)
    out[order] = y_ser
    return out


# revision 29
# speedup vs baseline: 1.0271x; 1.0271x over previous
"""Trainium2 Bass kernel for a serialized-attention transformer block.

Strategy (8 NeuronCores, data-parallel over serialized patches):
  host: fold LoRA + LN affine into weights, gather rows into serialized
        order, transpose to feature-major [C, rows], shard rows 8 ways.
  device (per core, rows R=8192, all activations feature-major):
        LN1 -> qkv -> per-patch attention (128-row patches) -> proj
        -> residual -> LN2 -> mlp (gelu-tanh) -> residual.
  host: transpose back and scatter rows to original order.

LN statistics are computed with ones-matmuls on the tensor engine
(partition-axis reductions); per-row scalars are broadcast across
partitions with K=1 matmuls.  Attention uses transposed scores
(lhsT=k, rhs=q) so softmax denominators come from a ones-matmul and no
PE transposes are needed anywhere.
"""

import os
import sys

import numpy as np

if "/opt/trn_rl_repo" not in sys.path:
    sys.path.insert(0, "/opt/trn_rl_repo")

N, C, H, K, R = 65536, 512, 8, 128, 16
D = C // H
HID = 4 * C
LORA_SCALE = 32.0 / 16.0
SCALE = D**-0.5
NCORES = 8
RPC = N // NCORES          # rows per core
RT = 512                   # rows per tile (4 patches)
NRT = RPC // RT
PPT = RT // K              # patches per row-tile
CCH = C // 128             # feature chunks of x (4)
QKCH = 8                   # q+k feature chunks
HCH = HID // 128           # hidden chunks (16)
F32 = None                 # set at build time (mybir.dt.float32)

_STATE = {}


def _build():
    import concourse.tile as tile
    from concourse import bacc, mybir

    nrt = int(os.environ.get("KERNEL_NRT", NRT))
    passes = os.environ.get("KERNEL_PASSES", "AB")
    f32 = mybir.dt.float32
    nc = bacc.Bacc(None, target_bir_lowering=False, debug=False)

    xin = nc.dram_tensor("xin", [C, RPC], f32, kind="ExternalInput")
    yout = nc.dram_tensor("yout", [C, RPC], f32, kind="ExternalOutput")
    f2d = nc.dram_tensor("feat2", [C, RPC], f32, kind="Internal")

    wqkv = nc.dram_tensor("wqkv", [128, CCH, 3 * C], f32, kind="ExternalInput")
    bqkv = nc.dram_tensor("bqkv", [128, 12], f32, kind="ExternalInput")
    bvbc = nc.dram_tensor("bvbc", [C], f32, kind="ExternalInput")
    wproj = nc.dram_tensor("wproj", [128, CCH, C], f32, kind="ExternalInput")
    bproj = nc.dram_tensor("bproj", [128, CCH], f32, kind="ExternalInput")
    w1 = nc.dram_tensor("w1", [128, CCH, HID], f32, kind="ExternalInput")
    b1h = nc.dram_tensor("b1h", [128, HCH], f32, kind="ExternalInput")
    w2 = nc.dram_tensor("w2", [128, HCH, C], f32, kind="ExternalInput")
    b2o = nc.dram_tensor("b2o", [128, CCH], f32, kind="ExternalInput")

    import concourse.bass as bass

    xin_r = xin[:].rearrange("(c p) r -> p c r", p=128)
    yout_r = yout[:].rearrange("(c p) r -> p c r", p=128)
    f2d_r = f2d[:].rearrange("(c p) r -> p c r", p=128)

    with tile.TileContext(nc) as tc:
        with (
            tc.tile_pool(name="const", bufs=1) as constp,
            tc.tile_pool(name="psum", bufs=1, space="PSUM") as psp,
        ):
            ones128 = constp.tile([128, 1], bf16)
            nc.vector.memset(ones128, 1.0)
            invC = constp.tile([128, 1], f32)
            nc.vector.memset(invC, 1.0 / C)
            ones_row = constp.tile([1, 128], f32)
            nc.vector.memset(ones_row, 1.0)
            epsb = constp.tile([128, 1], f32)
            nc.vector.memset(epsb, 1e-5)

            def layernorm(sb, x, xh):
                """x: [128, CCH, RT] sbuf -> xh normalized (no affine)."""
                x2 = sb.tile([128, CCH, RT], f32, tag="x2", bufs=1)
                nc.vector.tensor_mul(x2[:], x[:], x[:])
                s1 = psp.tile([1, RT], f32, tag="pss", bufs=2)
                s2 = psp.tile([1, RT], f32, tag="pss", bufs=2)
                for c in range(CCH):
                    nc.tensor.matmul(
                        s1[:], invC[:], x[:, c, :], start=(c == 0), stop=(c == CCH - 1)
                    )
                for c in range(CCH):
                    nc.tensor.matmul(
                        s2[:], invC[:], x2[:, c, :], start=(c == 0), stop=(c == CCH - 1)
                    )
                s1b = sb.tile([1, RT], f32, tag="s1b", bufs=2)
                s2b = sb.tile([1, RT], f32, tag="s2b", bufs=2)
                nc.scalar.copy(s1b[:], s1[:])
                nc.scalar.copy(s2b[:], s2[:])
                mb = psp.tile([128, RT], f32, tag="psb", bufs=6)
                m2b = psp.tile([128, RT], f32, tag="psb", bufs=6)
                nc.tensor.matmul(mb[:], ones_row[:], s1b[:], start=True, stop=True)
                nc.tensor.matmul(m2b[:], ones_row[:], s2b[:], start=True, stop=True)
                var = sb.tile([128, RT], f32, tag="var", bufs=2)
                nc.scalar.square(var[:], mb[:])
                nc.vector.tensor_sub(var[:], m2b[:], var[:])
                sd = sb.tile([128, RT], f32, tag="sd", bufs=2)
                nc.scalar.activation(
                    sd[:], var[:], mybir.ActivationFunctionType.Sqrt, bias=epsb[:]
                )
                ab = sb.tile([128, RT], f32, tag="ab", bufs=2)
                nc.vector.reciprocal(ab[:], sd[:])
                mbb = mb[:, None, :].to_broadcast([128, CCH, RT])
                abb = ab[:, None, :].to_broadcast([128, CCH, RT])
                nc.vector.tensor_sub(xh[:], x[:], mbb)
                nc.vector.tensor_mul(xh[:], xh[:], abb)

            # ---------------- pass A: attention block ----------------
            if "A" not in passes:
                nrt_a = 0
            else:
                nrt_a = nrt
            with (
                tc.tile_pool(name="wA", bufs=1) as wp,
                tc.tile_pool(name="sbA", bufs=1) as sb,
            ):
                wqkv_sb = wp.tile([128, CCH, 3 * C], f32)
                nc.gpsimd.dma_start(wqkv_sb[:], wqkv[:])
                bqkv_sb = wp.tile([128, 12], f32)
                nc.gpsimd.dma_start(bqkv_sb[:], bqkv[:])
                wproj_sb = wp.tile([128, CCH, C], f32)
                nc.gpsimd.dma_start(wproj_sb[:], wproj[:])
                bproj_sb = wp.tile([128, CCH], f32)
                nc.gpsimd.dma_start(bproj_sb[:], bproj[:])
                bv_sb = wp.tile([128, C], f32)
                nc.gpsimd.dma_start(
                    bv_sb[:],
                    bass.AP(tensor=bvbc, offset=0, ap=[[0, 128], [1, C]]),
                )

                for rt in range(nrt_a):
                    rsl = slice(rt * RT, (rt + 1) * RT)
                    x = sb.tile([128, CCH, RT], f32, tag="x", bufs=3)
                    nc.sync.dma_start(x[:], xin_r[:, :, rsl])
                    xh = sb.tile([128, CCH, RT], f32, tag="xh", bufs=2)
                    layernorm(sb, x, xh)

                    # q, k (feature-major) with bias
                    q = sb.tile([128, CCH, RT], bf16, tag="q", bufs=2)
                    k = sb.tile([128, CCH, RT], bf16, tag="k", bufs=2)
                    for fc in range(QKCH):
                        ps = psp.tile([128, RT], f32, tag="psb", bufs=6)
                        for c in range(CCH):
                            nc.tensor.matmul(
                                ps[:],
                                wqkv_sb[:, c, fc * 128 : (fc + 1) * 128],
                                xh[:, c, :],
                                start=(c == 0),
                                stop=(c == CCH - 1),
                            )
                        if fc < CCH:
                            # query bias kept (scaled); key bias provably
                            # cancels in softmax (per-query constant), so k
                            # eviction is a plain copy on the scalar engine.
                            nc.vector.tensor_scalar(
                                q[:, fc, :],
                                ps[:],
                                bqkv_sb[:, fc : fc + 1],
                                None,
                                mybir.AluOpType.add,
                            )
                        else:
                            nc.scalar.copy(k[:, fc - CCH, :], ps[:])
                    # v (row-major per patch) with bias
                    v = sb.tile([128, PPT, H, D], bf16, tag="v", bufs=2)
                    for pi in range(PPT):
                        psl = slice(pi * K, (pi + 1) * K)
                        psv = psp.tile([128, C], f32, tag="psb", bufs=6)
                        for c in range(CCH):
                            nc.tensor.matmul(
                                psv[:],
                                xh[:, c, psl],
                                wqkv_sb[:, c, 2 * C : 3 * C],
                                start=(c == 0),
                                stop=(c == CCH - 1),
                            )
                        nc.vector.tensor_add(
                            v[:, pi, :, :].rearrange("p h d -> p (h d)"),
                            psv[:],
                            bv_sb[:],
                        )

                    # attention per patch
                    o = sb.tile([128, CCH, PPT, K], f32, tag="o", bufs=2)
                    for pi in range(PPT):
                        psl = slice(pi * K, (pi + 1) * K)
                        sa = psp.tile([128, CCH, K], f32, tag="psb", bufs=6)
                        sbp = psp.tile([128, CCH, K], f32, tag="psb", bufs=6)
                        for j in range(CCH):
                            nc.tensor.matmul(
                                sa[:, j, :],
                                k[0:64, j, psl],
                                q[0:64, j, psl],
                                start=True,
                                stop=True,
                            )
                            nc.tensor.matmul(
                                sbp[:, j, :],
                                k[64:128, j, psl],
                                q[64:128, j, psl],
                                start=True,
                                stop=True,
                            )
                        ea = sb.tile([128, CCH, K], bf16, tag="ea", bufs=3)
                        eb = sb.tile([128, CCH, K], bf16, tag="eb", bufs=3)
                        nc.scalar.activation(
                            ea[:], sa[:], mybir.ActivationFunctionType.Exp
                        )
                        nc.scalar.activation(
                            eb[:], sbp[:], mybir.ActivationFunctionType.Exp
                        )
                        sua = psp.tile([1, RT], f32, tag="pss", bufs=2)
                        sub = psp.tile([1, RT], f32, tag="pss", bufs=2)
                        nc.tensor.matmul(
                            sua[:], ones128[:], ea[:].rearrange("p c r -> p (c r)"),
                            start=True, stop=True,
                        )
                        nc.tensor.matmul(
                            sub[:], ones128[:], eb[:].rearrange("p c r -> p (c r)"),
                            start=True, stop=True,
                        )
                        ra = sb.tile([1, RT], f32, tag="ra", bufs=2)
                        rb = sb.tile([1, RT], f32, tag="rb", bufs=2)
                        with nc.allow_low_precision(reason="f32r recip for matmul"):
                            nc.vector.reciprocal(ra[:], sua[:])
                            nc.vector.reciprocal(rb[:], sub[:])
                        rba = psp.tile([128, CCH, K], f32, tag="psb", bufs=6)
                        rbb = psp.tile([128, CCH, K], f32, tag="psb", bufs=6)
                        nc.tensor.matmul(
                            rba[:].rearrange("p c r -> p (c r)"), ones_row[:], ra[:],
                            start=True, stop=True,
                        )
                        nc.tensor.matmul(
                            rbb[:].rearrange("p c r -> p (c r)"), ones_row[:], rb[:],
                            start=True, stop=True,
                        )
                        ops = psp.tile([128, CCH, K], f32, tag="psb", bufs=6)
                        for j in range(CCH):
                            nc.tensor.matmul(
                                ops[0:64, j, :],
                                v[:, pi, 2 * j, :],
                                ea[:, j, :],
                                start=True,
                                stop=True,
                            )
                            nc.tensor.matmul(
                                ops[64:128, j, :],
                                v[:, pi, 2 * j + 1, :],
                                eb[:, j, :],
                                start=True,
                                stop=True,
                            )
                        nc.vector.tensor_mul(
                            o[0:64, :, pi, :], ops[0:64, :, :], rba[0:64, :, :]
                        )
                        nc.vector.tensor_mul(
                            o[64:128, :, pi, :], ops[64:128, :, :], rbb[64:128, :, :]
                        )

                    if rt + 1 < nrt_a:
                        xh_c = ln_finish(sb, x_c, *st_c)

                    # proj + residual -> feat2
                    f2 = sb.tile([128, CCH, RT], f32, tag="f2", bufs=2)
                    for c in range(CCH):
                        ps = psp.tile([128, RT], f32, tag="psb", bufs=6)
                        for cc in range(CCH):
                            nc.tensor.matmul(
                                ps[:],
                                wproj_sb[:, cc, c * 128 : (c + 1) * 128],
                                o[:, cc, :, :].rearrange("p t r -> p (t r)"),
                                start=(cc == 0),
                                stop=(cc == CCH - 1),
                            )
                        nc.vector.tensor_scalar(
                            f2[:, c, :],
                            ps[:],
                            bproj_sb[:, c : c + 1],
                            None,
                            mybir.AluOpType.add,
                        )
                        nc.vector.tensor_add(f2[:, c, :], f2[:, c, :], x[:, c, :])
                    nc.gpsimd.dma_start(f2d_r[:, :, rsl], f2[:])

            # ---------------- pass B: MLP block ----------------
            if "B" not in passes:
                nrt_b = 0
            else:
                nrt_b = nrt
            with (
                tc.tile_pool(name="wB", bufs=1) as wp,
                tc.tile_pool(name="sbB", bufs=1) as sb,
            ):
                w1_sb = wp.tile([128, CCH, HID], f32)
                nc.gpsimd.dma_start(w1_sb[:], w1[:])
                b1h_sb = wp.tile([128, HCH], f32)
                nc.gpsimd.dma_start(b1h_sb[:], b1h[:])
                w2_sb = wp.tile([128, HCH, C], f32)
                nc.gpsimd.dma_start(w2_sb[:], w2[:])
                b2o_sb = wp.tile([128, CCH], f32)
                nc.gpsimd.dma_start(b2o_sb[:], b2o[:])

                for rt in range(nrt_b):
                    rsl = slice(rt * RT, (rt + 1) * RT)
                    x = sb.tile([128, CCH, RT], f32, tag="x", bufs=3)
                    nc.sync.dma_start(x[:], f2d_r[:, :, rsl])
                    xh = sb.tile([128, CCH, RT], f32, tag="xh", bufs=2)
                    layernorm(sb, x, xh)

                    h = sb.tile([128, HCH, RT], f32, tag="h", bufs=1)
                    for fc in range(HCH):
                        ps = psp.tile([128, RT], f32, tag="psb", bufs=6)
                        for c in range(CCH):
                            nc.tensor.matmul(
                                ps[:],
                                w1_sb[:, c, fc * 128 : (fc + 1) * 128],
                                xh[:, c, :],
                                start=(c == 0),
                                stop=(c == CCH - 1),
                            )
                        nc.scalar.activation(
                            h[:, fc, :],
                            ps[:],
                            mybir.ActivationFunctionType.Gelu_apprx_tanh,
                            bias=b1h_sb[:, fc : fc + 1],
                        )
                    yo = sb.tile([128, CCH, RT], f32, tag="yo", bufs=2)
                    for c in range(CCH):
                        ps = psp.tile([128, RT], f32, tag="psb", bufs=6)
                        for cc in range(HCH):
                            nc.tensor.matmul(
                                ps[:],
                                w2_sb[:, cc, c * 128 : (c + 1) * 128],
                                h[:, cc, :],
                                start=(cc == 0),
                                stop=(cc == HCH - 1),
                            )
                        nc.vector.tensor_scalar(
                            yo[:, c, :],
                            ps[:],
                            b2o_sb[:, c : c + 1],
                            None,
                            mybir.AluOpType.add,
                        )
                        nc.vector.tensor_add(yo[:, c, :], yo[:, c, :], x[:, c, :])
                    nc.gpsimd.dma_start(yout_r[:, :, rsl], yo[:])

    nc.compile()
    return nc


def _fold_weights(ins):
    """Host-side constant folding: LoRA into base weights, LN affine into
    the following linear layer, attention scale into q columns."""
    g = lambda n: np.asarray(ins[n], np.float32)
    out = {}

    weff = g("Wqkv") + LORA_SCALE * (g("Aqkv") @ g("Bqkv"))
    wq = g("g1")[:, None] * weff
    bq = g("bqkv") + g("b1") @ weff
    wq[:, :C] *= SCALE
    bq = bq.copy()
    bq[:C] *= SCALE
    out["wqkv"] = np.ascontiguousarray(
        wq.reshape(CCH, 128, 3 * C).transpose(1, 0, 2)
    )
    out["bqkv"] = np.ascontiguousarray(bq.reshape(12, 128).T)
    out["bvbc"] = np.ascontiguousarray(bq[2 * C : 3 * C])

    wp = g("Wproj") + LORA_SCALE * (g("Aproj") @ g("Bproj"))
    out["wproj"] = np.ascontiguousarray(wp.reshape(CCH, 128, C).transpose(1, 0, 2))
    out["bproj"] = np.ascontiguousarray(g("bproj").reshape(CCH, 128).T)

    w1eff = g("W1") + LORA_SCALE * (g("A1") @ g("B1"))
    w1f = g("g2")[:, None] * w1eff
    b1f = g("bfc1") + g("b2") @ w1eff
    out["w1"] = np.ascontiguousarray(w1f.reshape(CCH, 128, HID).transpose(1, 0, 2))
    out["b1h"] = np.ascontiguousarray(b1f.reshape(HCH, 128).T)

    w2eff = g("W2") + LORA_SCALE * (g("A2") @ g("B2"))
    out["w2"] = np.ascontiguousarray(w2eff.reshape(HCH, 128, C).transpose(1, 0, 2))
    out["b2o"] = np.ascontiguousarray(g("bfc2").reshape(CCH, 128).T)
    return out


def kernel(**inputs):
    from concourse.bass_utils import run_bass_kernel_spmd

    if "nc" not in _STATE:
        _STATE["nc"] = _build()
    nc = _STATE["nc"]

    feat = np.asarray(inputs["feat"], np.float32)
    order = np.asarray(inputs["order"], np.int64)
    w = _fold_weights(inputs)

    feat_ser = feat[order]  # serialized order
    in_maps = []
    for cid in range(NCORES):
        m = dict(w)
        m["xin"] = np.ascontiguousarray(
            feat_ser[cid * RPC : (cid + 1) * RPC].T
        )
        in_maps.append(m)

    res = run_bass_kernel_spmd(nc, in_maps, core_ids=list(range(NCORES)))
    _STATE["last_result"] = res

    y_ser = np.empty((N, C), np.float32)
    for cid in range(NCORES):
        y_ser[cid * RPC : (cid + 1) * RPC] = res.results[cid]["yout"].T
    out = np.empty((N, C), np.float32)
    out[order] = y_ser
    return out
